# revision 6
# baseline (speedup 1.0000x reference)
"""CTC boundary loss v3 kernel for 8 Trainium2 NeuronCores.

Math (derived from the reference, which reduces to per-sample scalars):
  blank  = ctc_log_probs[:, :, 0]                      [B,T]
  trig   = (1.0 - blank) > log(3)                      [B,T]
  n_seg  = sum(trig * mask)  per sample                [B]
  rsum   = sum(alpha, axis=1)                          [B]
  len_i  = max(n_seg, 1)
  L      = min(max_i len_i, max_i text_length_i)
  c_i    = min(text_length_i, L)
  loss_i = min(n_seg_i, c_i) * |rsum_i - 1| + relu(c_i - len_i)
  out    = sum(loss_i) / B

Device (data parallel, 2 samples/core): strided gather of the blank
channel + per-sample reductions -> per-core [2, 2] stats (n_seg, rsum).
Host: gathers 16 stats and folds the O(B) scalar tail.
"""

import sys

import numpy as np

if "/opt/trn_rl_repo" not in sys.path:
    sys.path.insert(0, "/opt/trn_rl_repo")

import concourse.bass as bass
import concourse.mybir as mybir
import concourse.tile as tile
from concourse.bass_utils import run_bass_kernel_spmd

B, T, V = 16, 2048, 1024
N_CORES = 8
BPC = B // N_CORES            # samples per core = 2
P = 128                       # SBUF partitions
PPS = P // BPC                # partitions per sample = 64
KC = T // PPS                 # free-dim cols per partition = 32
LOG_THR = float(np.log(3.0))  # rounds to the same f32 the reference compares with

_CACHE = {}


def build_nc():
    """Raw bass (manual semaphores): single-wait-per-instruction codegen
    rejects Tile's multi-wait joins, and raw bass also skips Tile's
    end-of-kernel all-engine barriers."""
    f32 = mybir.dt.float32
    nc = bass.Bass()
    ctc = nc.dram_tensor("ctc", [BPC, T, V], f32, kind="ExternalInput")
    alpha = nc.dram_tensor("alpha", [BPC, T], f32, kind="ExternalInput")
    mask = nc.dram_tensor("mask", [BPC, T], f32, kind="ExternalInput")
    out = nc.dram_tensor("out", [BPC, 2], f32, kind="ExternalOutput")

    with (
        nc.sbuf_tensor([P, KC], f32) as bt,        # blank log-probs, gathered
        nc.sbuf_tensor([P, KC], f32) as mt,        # mask
        nc.sbuf_tensor([P, 2 * KC], f32) as rhs,   # [spikes | alpha]
        nc.sbuf_tensor([P, BPC], f32) as seg,      # one-hot segment matrix
        nc.sbuf_tensor([BPC, 2], f32) as st,       # per-sample [n_seg, rsum]
        nc.psum_tensor([BPC, 2 * KC], f32) as ps,
        nc.semaphore("dma_sem") as dma_sem,
        nc.semaphore("dve_sem") as dve_sem,
        nc.semaphore("pe_sem") as pe_sem,
        nc.Block() as block,
    ):

        @block.sync
        def _(sync):
            # sample b -> partitions [b*PPS, (b+1)*PPS), t = p_local*KC + k
            with nc.allow_non_contiguous_dma(reason="blank-channel gather"):
                sync.dma_start(
                    out=bt[:, :],
                    in_=ctc[:, :, 0].rearrange("b (p k) -> (b p) k", k=KC),
                ).then_inc(dma_sem, 16)
            sync.dma_start(
                out=mt[:, :],
                in_=mask.rearrange("b (p k) -> (b p) k", k=KC),
            ).then_inc(dma_sem, 16)
            sync.dma_start(
                out=rhs[:, KC:],
                in_=alpha.rearrange("b (p k) -> (b p) k", k=KC),
            ).then_inc(dma_sem, 16)
            sync.wait_ge(dve_sem, 8)
            sync.dma_start(out=out[:, :], in_=st[:, :]).then_inc(dma_sem, 16)
            sync.wait_ge(dma_sem, 64)  # out DMA landed before NEFF completion

        @block.vector
        def _(vector):
            for b in range(BPC):
                for c in range(BPC):
                    vector.memset(
                        seg[b * PPS : (b + 1) * PPS, c : c + 1],
                        1.0 if b == c else 0.0,
                    ).then_inc(dve_sem, 1)  # -> 4
            vector.wait_ge(dma_sem, 48)
            # z = 1 - x with identical f32 rounding to the reference
            vector.tensor_scalar(
                bt[:, :], bt[:, :], -1.0, 1.0,
                mybir.AluOpType.mult, mybir.AluOpType.add,
            ).then_inc(dve_sem, 1)  # -> 5
            vector.wait_ge(dve_sem, 5)
            # trig = (z > log 3) -> 1.0/0.0
            vector.tensor_scalar(
                bt[:, :], bt[:, :], LOG_THR, None, mybir.AluOpType.is_gt,
            ).then_inc(dve_sem, 1)  # -> 6
            vector.wait_ge(dve_sem, 6)
            # spikes = trig * mask
            vector.tensor_tensor(
                rhs[:, 0:KC], bt[:, :], mt[:, :], mybir.AluOpType.mult,
            ).then_inc(dve_sem, 1)  # -> 7
            vector.wait_ge(pe_sem, 1)
            # reduce the KC partials: st[s, 0] = n_seg_s, st[s, 1] = rsum_s
            vector.tensor_reduce(
                st[:, :],
                ps[:, :].rearrange("s (h k) -> s h k", k=KC),
                mybir.AxisListType.X,
                mybir.AluOpType.add,
            ).then_inc(dve_sem, 1)  # -> 8

        @block.tensor
        def _(tensor):
            tensor.wait_ge(dve_sem, 7)
            # ps[s, j] = sum_p seg[p, s] * rhs[p, j] (segmented partition sum)
            tensor.matmul(
                ps[:, :], seg[:, :], rhs[:, :], start=True, stop=True
            ).then_inc(pe_sem, 1)

    return nc


def _device_stats(ctc_log_probs, alpha, mask, trace=False):
    """Run the SPMD bass kernel; returns (n_seg[B], rsum[B], exec_time_ns)."""
    if "nc" not in _CACHE:
        _CACHE["nc"] = build_nc()
    nc = _CACHE["nc"]

    in_maps = []
    for i in range(N_CORES):
        s = slice(i * BPC, (i + 1) * BPC)
        in_maps.append(
            {
                "ctc": np.ascontiguousarray(ctc_log_probs[s], dtype=np.float32),
                "alpha": np.ascontiguousarray(alpha[s], dtype=np.float32),
                "mask": np.ascontiguousarray(mask[s], dtype=np.float32),
            }
        )
    res = run_bass_kernel_spmd(nc, in_maps, list(range(N_CORES)), trace=trace)
    stats = np.concatenate([np.asarray(r["out"]) for r in res.results], axis=0)
    return stats[:, 0], stats[:, 1], res.exec_time_ns


def _tail(n_seg, rsum, text_length):
    """O(B) scalar tail: combine per-sample stats into the loss."""
    n_seg = n_seg.astype(np.float64)
    rsum = rsum.astype(np.float64)
    text = np.asarray(text_length).astype(np.float64)
    len_i = np.maximum(n_seg, 1.0)
    L = min(len_i.max(), text.max())
    c = np.minimum(text, L)
    loss = np.minimum(n_seg, c) * np.abs(rsum - 1.0) + np.maximum(c - len_i, 0.0)
    return np.float32(loss.sum() / n_seg.shape[0])


def kernel(alpha, ctc_log_probs, mask, text_length):
    alpha = np.asarray(alpha)
    ctc_log_probs = np.asarray(ctc_log_probs)
    mask = np.asarray(mask)
    text_length = np.asarray(text_length)
    n_seg, rsum, _ = _device_stats(ctc_log_probs, alpha, mask)
    return _tail(n_seg, rsum, text_length)


# revision 15
# speedup vs baseline: 1.1836x; 1.1836x over previous
"""CTC boundary loss v3 kernel for 8 Trainium2 NeuronCores.

Math (derived from the reference, which reduces to per-sample scalars):
  blank  = ctc_log_probs[:, :, 0]                      [B,T]
  trig   = (1.0 - blank) > log(3)                      [B,T]
  n_seg  = sum(trig * mask)  per sample                [B]
  rsum   = sum(alpha, axis=1)                          [B]
  len_i  = max(n_seg, 1)
  L      = min(max_i len_i, max_i text_length_i)
  c_i    = min(text_length_i, L)
  loss_i = min(n_seg_i, c_i) * |rsum_i - 1| + relu(c_i - len_i)
  out    = sum(loss_i) / B

Device (data parallel, 2 samples/core): strided gather of the blank
channel + per-sample reductions -> per-core [2, 2] stats (n_seg, rsum).
Host: gathers 16 stats and folds the O(B) scalar tail.
"""

import sys

import numpy as np

if "/opt/trn_rl_repo" not in sys.path:
    sys.path.insert(0, "/opt/trn_rl_repo")

import concourse.bass as bass
import concourse.mybir as mybir
import concourse.tile as tile
from concourse.bass_utils import run_bass_kernel_spmd

B, T, V = 16, 2048, 1024
N_CORES = 8
BPC = B // N_CORES            # samples per core = 2
P = 128                       # SBUF partitions
PPS = P // BPC                # partitions per sample = 64
KC = T // PPS                 # free-dim cols per partition = 32
LOG_THR = float(np.log(3.0))  # rounds to the same f32 the reference compares with

_CACHE = {}


def build_nc():
    """Raw bass (manual semaphores): single-wait-per-instruction codegen
    rejects Tile's multi-wait joins, and raw bass also skips Tile's
    end-of-kernel all-engine barriers.

    Layout: sample b -> partitions [b*64, (b+1)*64), t = p_local*32 + k.
    The blank-channel gather (4096 4-byte strided descriptors) is split
    across both HWDGE rings (SP + ACT) to halve serial descriptor issue.
    Per-partition partials land in red[128, 2] (spike counts | alpha
    sums); one tiny TensorE matmul against a one-hot segment matrix does
    the segmented partition reduction to [2, 2]."""
    f32 = mybir.dt.float32
    nc = bass.Bass(enable_partition_id=False)
    ctc = nc.dram_tensor("ctc", [BPC, T, V], f32, kind="ExternalInput")
    alpha = nc.dram_tensor("alpha", [BPC, T], f32, kind="ExternalInput")
    mask = nc.dram_tensor("mask", [BPC, T], f32, kind="ExternalInput")
    out = nc.dram_tensor("out", [BPC, 2], f32, kind="ExternalOutput")

    H = KC // 2
    # block layout: sample b -> partitions [b*PPS, (b+1)*PPS), t = p_local*KC + k
    gsrc = ctc[:, :, 0].rearrange("b (p k) -> (b p) k", k=KC)
    asrc = alpha.rearrange("b (p k) -> (b p) k", k=KC)
    msrc = mask.rearrange("b (p k) -> (b p) k", k=KC)

    with (
        nc.sbuf_tensor([P, KC], f32) as bt,    # blank log-probs, gathered
        nc.sbuf_tensor([P, KC], f32) as mt,    # mask
        nc.sbuf_tensor([P, KC], f32) as at,    # alpha
        nc.sbuf_tensor([P, KC], f32) as jt,    # spikes scratch
        nc.sbuf_tensor([P, 2], f32) as red,    # [spike partials | alpha partials]
        nc.sbuf_tensor([P, BPC], f32) as seg,  # one-hot segment matrix
        nc.sbuf_tensor([BPC, 2], f32) as st,   # per-sample [n_seg, rsum]
        nc.psum_tensor([BPC, 2], f32) as ps,
        nc.semaphore("ga_sem") as ga_sem,
        nc.semaphore("gb_sem") as gb_sem,
        nc.semaphore("al_sem") as al_sem,
        nc.semaphore("mk_sem") as mk_sem,
        nc.semaphore("out_sem") as out_sem,
        nc.semaphore("dve_sem") as dve_sem,
        nc.semaphore("pe_sem") as pe_sem,
        nc.Block() as block,
    ):

        @block.sync
        def _(sync):
            with nc.allow_non_contiguous_dma(reason="blank-channel gather"):
                sync.dma_start(out=bt[:, 0:H], in_=gsrc[:, 0:H]).then_inc(ga_sem, 16)
            sync.dma_start(out=at[:, :], in_=asrc[:, :]).then_inc(al_sem, 16)
            sync.wait_ge(dve_sem, 8)
            sync.dma_start(out=out[:, :], in_=st[:, :]).then_inc(out_sem, 16)
            sync.wait_ge(out_sem, 16)  # out DMA landed before NEFF completion

        @block.scalar
        def _(scalar):
            with nc.allow_non_contiguous_dma(reason="blank-channel gather"):
                scalar.dma_start(out=bt[:, H:KC], in_=gsrc[:, H:KC]).then_inc(
                    gb_sem, 16
                )
            scalar.dma_start(out=mt[:, :], in_=msrc[:, :]).then_inc(mk_sem, 16)

        @block.vector
        def _(vector):
            for b in range(BPC):
                for c in range(BPC):
                    vector.memset(
                        seg[b * PPS : (b + 1) * PPS, c : c + 1],
                        1.0 if b == c else 0.0,
                    ).then_inc(dve_sem, 1)  # -> 4
            vector.wait_ge(ga_sem, 16)
            vector.wait_ge(gb_sem, 16)
            # z = 1 - x with identical f32 rounding to the reference
            vector.tensor_scalar(
                bt[:, :], bt[:, :], -1.0, 1.0,
                mybir.AluOpType.mult, mybir.AluOpType.add,
            ).then_inc(dve_sem, 1)  # -> 5
            vector.wait_ge(mk_sem, 16)  # mask loaded
            vector.wait_ge(dve_sem, 5)
            # spikes = (z > log 3) * mask; accum_out = per-partition counts
            vector.scalar_tensor_tensor(
                jt[:, :], bt[:, :], LOG_THR, mt[:, :],
                mybir.AluOpType.is_gt, mybir.AluOpType.mult,
                accum_out=red[:, 0:1],
            ).then_inc(dve_sem, 1)  # -> 6
            vector.wait_ge(al_sem, 16)  # alpha loaded
            vector.tensor_reduce(
                red[:, 1:2], at[:, :], mybir.AxisListType.X, mybir.AluOpType.add,
            ).then_inc(dve_sem, 1)  # -> 7
            vector.wait_ge(pe_sem, 1)
            vector.tensor_copy(st[:, :], ps[:, :]).then_inc(dve_sem, 1)  # -> 8

        @block.tensor
        def _(tensor):
            tensor.wait_ge(dve_sem, 7)
            # ps[s, j] = sum_p seg[p, s] * red[p, j] (segmented partition sum)
            tensor.matmul(
                ps[:, :], seg[:, :], red[:, :], start=True, stop=True
            ).then_inc(pe_sem, 1)

    return nc


def _device_stats(ctc_log_probs, alpha, mask, trace=False, return_res=False):
    """Run the SPMD bass kernel; returns (n_seg[B], rsum[B], exec_time_ns)."""
    if "nc" not in _CACHE:
        _CACHE["nc"] = build_nc()
    nc = _CACHE["nc"]

    in_maps = []
    for i in range(N_CORES):
        s = slice(i * BPC, (i + 1) * BPC)
        in_maps.append(
            {
                "ctc": np.ascontiguousarray(ctc_log_probs[s], dtype=np.float32),
                "alpha": np.ascontiguousarray(alpha[s], dtype=np.float32),
                "mask": np.ascontiguousarray(mask[s], dtype=np.float32),
            }
        )
    res = run_bass_kernel_spmd(nc, in_maps, list(range(N_CORES)), trace=trace)
    stats = np.concatenate([np.asarray(r["out"]) for r in res.results], axis=0)
    if return_res:
        return stats[:, 0], stats[:, 1], res.exec_time_ns, res
    return stats[:, 0], stats[:, 1], res.exec_time_ns


def _tail(n_seg, rsum, text_length):
    """O(B) scalar tail: combine per-sample stats into the loss."""
    n_seg = n_seg.astype(np.float64)
    rsum = rsum.astype(np.float64)
    text = np.asarray(text_length).astype(np.float64)
    len_i = np.maximum(n_seg, 1.0)
    L = min(len_i.max(), text.max())
    c = np.minimum(text, L)
    loss = np.minimum(n_seg, c) * np.abs(rsum - 1.0) + np.maximum(c - len_i, 0.0)
    return np.float32(loss.sum() / n_seg.shape[0])


def kernel(alpha, ctc_log_probs, mask, text_length):
    alpha = np.asarray(alpha)
    ctc_log_probs = np.asarray(ctc_log_probs)
    mask = np.asarray(mask)
    text_length = np.asarray(text_length)
    n_seg, rsum, _ = _device_stats(ctc_log_probs, alpha, mask)
    return _tail(n_seg, rsum, text_length)


# revision 16
# speedup vs baseline: 1.1849x; 1.0011x over previous
"""CTC boundary loss v3 kernel for 8 Trainium2 NeuronCores.

Math (derived from the reference, which reduces to per-sample scalars):
  blank  = ctc_log_probs[:, :, 0]                      [B,T]
  trig   = (1.0 - blank) > log(3)                      [B,T]
  n_seg  = sum(trig * mask)  per sample                [B]
  rsum   = sum(alpha, axis=1)                          [B]
  len_i  = max(n_seg, 1)
  L      = min(max_i len_i, max_i text_length_i)
  c_i    = min(text_length_i, L)
  loss_i = min(n_seg_i, c_i) * |rsum_i - 1| + relu(c_i - len_i)
  out    = sum(loss_i) / B

Device (data parallel, 2 samples/core): strided gather of the blank
channel + per-sample reductions -> per-core [2, 2] stats (n_seg, rsum).
Host: gathers 16 stats and folds the O(B) scalar tail.
"""

import sys

import numpy as np

if "/opt/trn_rl_repo" not in sys.path:
    sys.path.insert(0, "/opt/trn_rl_repo")

import concourse.bass as bass
import concourse.mybir as mybir
from concourse.bass_utils import run_bass_kernel_spmd

B, T, V = 16, 2048, 1024
N_CORES = 8
BPC = B // N_CORES            # samples per core = 2
P = 128                       # SBUF partitions
PPS = P // BPC                # partitions per sample = 64
KC = T // PPS                 # free-dim cols per partition = 32
LOG_THR = float(np.log(3.0))
# Boundary constant: for every float32 x (incl. +-inf, NaN),
#   (float32(1.0) - x) > float32(LOG_THR)   <=>   x < TRIG_C
# (verified exhaustively around the flip point; it is 2 ulps away from the
# naive 1 - LOG_THR, so the comparison must use this exact constant).
TRIG_C = float(np.float32(-0.09861236810684204))

_CACHE = {}


def build_nc():
    """Raw bass (manual semaphores): single-wait-per-instruction codegen
    rejects Tile's multi-wait joins, and raw bass also skips Tile's
    end-of-kernel all-engine barriers.

    Layout: sample b -> partitions [b*64, (b+1)*64), t = p_local*32 + k.
    The blank-channel gather (4096 4-byte strided descriptors) is split
    across both HWDGE rings (SP + ACT) to halve serial descriptor issue.
    Per-partition partials land in red[128, 2] (spike counts | alpha
    sums); one tiny TensorE matmul against a one-hot segment matrix does
    the segmented partition reduction to [2, 2]."""
    f32 = mybir.dt.float32
    nc = bass.Bass(enable_partition_id=False)
    ctc = nc.dram_tensor("ctc", [BPC, T, V], f32, kind="ExternalInput")
    alpha = nc.dram_tensor("alpha", [BPC, T], f32, kind="ExternalInput")
    mask = nc.dram_tensor("mask", [BPC, T], f32, kind="ExternalInput")
    out = nc.dram_tensor("out", [BPC, 2], f32, kind="ExternalOutput")

    H = KC // 2
    # block layout: sample b -> partitions [b*PPS, (b+1)*PPS), t = p_local*KC + k
    gsrc = ctc[:, :, 0].rearrange("b (p k) -> (b p) k", k=KC)
    asrc = alpha.rearrange("b (p k) -> (b p) k", k=KC)
    msrc = mask.rearrange("b (p k) -> (b p) k", k=KC)

    with (
        nc.sbuf_tensor([P, KC], f32) as bt,    # blank log-probs, gathered
        nc.sbuf_tensor([P, KC], f32) as mt,    # mask
        nc.sbuf_tensor([P, KC], f32) as at,    # alpha
        nc.sbuf_tensor([P, KC], f32) as jt,    # spikes scratch
        nc.sbuf_tensor([P, 2], f32) as red,    # [spike partials | alpha partials]
        nc.sbuf_tensor([P, BPC], f32) as seg,  # one-hot segment matrix
        nc.sbuf_tensor([BPC, 2], f32) as st,   # per-sample [n_seg, rsum]
        nc.psum_tensor([BPC, 2], f32) as ps,
        nc.semaphore("ga_sem") as ga_sem,
        nc.semaphore("gb_sem") as gb_sem,
        nc.semaphore("al_sem") as al_sem,
        nc.semaphore("mk_sem") as mk_sem,
        nc.semaphore("out_sem") as out_sem,
        nc.semaphore("dve_sem") as dve_sem,
        nc.semaphore("pe_sem") as pe_sem,
        nc.Block() as block,
    ):

        @block.sync
        def _(sync):
            sync.dma_start(out=at[:, :], in_=asrc[:, :]).then_inc(al_sem, 16)
            with nc.allow_non_contiguous_dma(reason="blank-channel gather"):
                sync.dma_start(out=bt[:, 0:H], in_=gsrc[:, 0:H]).then_inc(ga_sem, 16)
            sync.wait_ge(dve_sem, 7)
            sync.dma_start(out=out[:, :], in_=st[:, :]).then_inc(out_sem, 16)
            sync.wait_ge(out_sem, 16)  # out DMA landed before NEFF completion

        @block.scalar
        def _(scalar):
            scalar.dma_start(out=mt[:, :], in_=msrc[:, :]).then_inc(mk_sem, 16)
            with nc.allow_non_contiguous_dma(reason="blank-channel gather"):
                scalar.dma_start(out=bt[:, H:KC], in_=gsrc[:, H:KC]).then_inc(
                    gb_sem, 16
                )

        @block.vector
        def _(vector):
            for b in range(BPC):
                for c in range(BPC):
                    vector.memset(
                        seg[b * PPS : (b + 1) * PPS, c : c + 1],
                        1.0 if b == c else 0.0,
                    ).then_inc(dve_sem, 1)  # -> 4
            vector.wait_ge(al_sem, 16)  # alpha loaded (while gathers fly)
            vector.tensor_reduce(
                red[:, 1:2], at[:, :], mybir.AxisListType.X, mybir.AluOpType.add,
            ).then_inc(dve_sem, 1)  # -> 5
            vector.wait_ge(ga_sem, 16)
            vector.wait_ge(gb_sem, 16)
            vector.wait_ge(mk_sem, 16)
            # spikes = (x < TRIG_C) * mask; accum_out = per-partition counts
            vector.scalar_tensor_tensor(
                jt[:, :], bt[:, :], TRIG_C, mt[:, :],
                mybir.AluOpType.is_lt, mybir.AluOpType.mult,
                accum_out=red[:, 0:1],
            ).then_inc(dve_sem, 1)  # -> 6
            vector.wait_ge(pe_sem, 1)
            vector.tensor_copy(st[:, :], ps[:, :]).then_inc(dve_sem, 1)  # -> 7

        @block.tensor
        def _(tensor):
            tensor.wait_ge(dve_sem, 6)
            # ps[s, j] = sum_p seg[p, s] * red[p, j] (segmented partition sum)
            tensor.matmul(
                ps[:, :], seg[:, :], red[:, :], start=True, stop=True
            ).then_inc(pe_sem, 1)

    return nc


def _device_stats(ctc_log_probs, alpha, mask, trace=False, return_res=False):
    """Run the SPMD bass kernel; returns (n_seg[B], rsum[B], exec_time_ns)."""
    if "nc" not in _CACHE:
        _CACHE["nc"] = build_nc()
    nc = _CACHE["nc"]

    in_maps = []
    for i in range(N_CORES):
        s = slice(i * BPC, (i + 1) * BPC)
        in_maps.append(
            {
                "ctc": np.ascontiguousarray(ctc_log_probs[s], dtype=np.float32),
                "alpha": np.ascontiguousarray(alpha[s], dtype=np.float32),
                "mask": np.ascontiguousarray(mask[s], dtype=np.float32),
            }
        )
    res = run_bass_kernel_spmd(nc, in_maps, list(range(N_CORES)), trace=trace)
    stats = np.concatenate([np.asarray(r["out"]) for r in res.results], axis=0)
    if return_res:
        return stats[:, 0], stats[:, 1], res.exec_time_ns, res
    return stats[:, 0], stats[:, 1], res.exec_time_ns


def _tail(n_seg, rsum, text_length):
    """O(B) scalar tail: combine per-sample stats into the loss."""
    n_seg = n_seg.astype(np.float64)
    rsum = rsum.astype(np.float64)
    text = np.asarray(text_length).astype(np.float64)
    len_i = np.maximum(n_seg, 1.0)
    L = min(len_i.max(), text.max())
    c = np.minimum(text, L)
    loss = np.minimum(n_seg, c) * np.abs(rsum - 1.0) + np.maximum(c - len_i, 0.0)
    return np.float32(loss.sum() / n_seg.shape[0])


def kernel(alpha, ctc_log_probs, mask, text_length):
    alpha = np.asarray(alpha)
    ctc_log_probs = np.asarray(ctc_log_probs)
    mask = np.asarray(mask)
    text_length = np.asarray(text_length)
    n_seg, rsum, _ = _device_stats(ctc_log_probs, alpha, mask)
    return _tail(n_seg, rsum, text_length)


# revision 17
# speedup vs baseline: 1.1927x; 1.0066x over previous
"""CTC boundary loss v3 kernel for 8 Trainium2 NeuronCores.

Math (derived from the reference, which reduces to per-sample scalars):
  blank  = ctc_log_probs[:, :, 0]                      [B,T]
  trig   = (1.0 - blank) > log(3)                      [B,T]
  n_seg  = sum(trig * mask)  per sample                [B]
  rsum   = sum(alpha, axis=1)                          [B]
  len_i  = max(n_seg, 1)
  L      = min(max_i len_i, max_i text_length_i)
  c_i    = min(text_length_i, L)
  loss_i = min(n_seg_i, c_i) * |rsum_i - 1| + relu(c_i - len_i)
  out    = sum(loss_i) / B

Device (data parallel, 2 samples/core): strided gather of the blank
channel + per-sample reductions -> per-core [2, 2] stats (n_seg, rsum).
Host: gathers 16 stats and folds the O(B) scalar tail.
"""

import sys

import numpy as np

if "/opt/trn_rl_repo" not in sys.path:
    sys.path.insert(0, "/opt/trn_rl_repo")

import concourse.bass as bass
import concourse.mybir as mybir
from concourse.bass_utils import run_bass_kernel_spmd

B, T, V = 16, 2048, 1024
N_CORES = 8
BPC = B // N_CORES            # samples per core = 2
P = 128                       # SBUF partitions
PPS = P // BPC                # partitions per sample = 64
KC = T // PPS                 # free-dim cols per partition = 32
LOG_THR = float(np.log(3.0))
# Boundary constant: for every float32 x (incl. +-inf, NaN),
#   (float32(1.0) - x) > float32(LOG_THR)   <=>   x < TRIG_C
# (verified exhaustively around the flip point; it is 2 ulps away from the
# naive 1 - LOG_THR, so the comparison must use this exact constant).
TRIG_C = float(np.float32(-0.09861236810684204))

_CACHE = {}


def build_nc():
    """Raw bass (manual semaphores): single-wait-per-instruction codegen
    rejects Tile's multi-wait joins, and raw bass also skips Tile's
    end-of-kernel all-engine barriers.

    Layout: sample b -> partitions [b*64, (b+1)*64), t = p_local*32 + k.
    The blank-channel gather (4096 4-byte strided descriptors) is split
    across both HWDGE rings (SP + ACT) to halve serial descriptor issue.
    Per-partition partials land in red[128, 2] (spike counts | alpha
    sums); one tiny TensorE matmul against a one-hot segment matrix does
    the segmented partition reduction to [2, 2]."""
    f32 = mybir.dt.float32
    nc = bass.Bass(enable_partition_id=False)
    ctc = nc.dram_tensor("ctc", [BPC, T, V], f32, kind="ExternalInput")
    alpha = nc.dram_tensor("alpha", [BPC, T], f32, kind="ExternalInput")
    mask = nc.dram_tensor("mask", [BPC, T], f32, kind="ExternalInput")
    out = nc.dram_tensor("out", [BPC, 2], f32, kind="ExternalOutput")

    H = KC // 2
    # block layout: sample b -> partitions [b*PPS, (b+1)*PPS), t = p_local*KC + k
    gsrc = ctc[:, :, 0].rearrange("b (p k) -> (b p) k", k=KC)
    asrc = alpha.rearrange("b (p k) -> (b p) k", k=KC)
    msrc = mask.rearrange("b (p k) -> (b p) k", k=KC)

    with (
        nc.sbuf_tensor([P, KC], f32) as bt,    # blank log-probs, gathered
        nc.sbuf_tensor([P, KC], f32) as mt,    # mask
        nc.sbuf_tensor([P, KC], f32) as at,    # alpha
        nc.sbuf_tensor([P, KC], f32) as jt,    # spikes scratch
        nc.sbuf_tensor([P, 2], f32) as red,    # [spike partials | alpha partials]
        nc.sbuf_tensor([P, BPC], f32) as seg,  # one-hot segment matrix
        nc.sbuf_tensor([BPC, 2], f32) as st,   # per-sample [n_seg, rsum]
        nc.psum_tensor([BPC, 2], f32) as ps,
        nc.semaphore("ga_sem") as ga_sem,
        nc.semaphore("gb_sem") as gb_sem,
        nc.semaphore("al_sem") as al_sem,
        nc.semaphore("mk_sem") as mk_sem,
        nc.semaphore("out_sem") as out_sem,
        nc.semaphore("dve_sem") as dve_sem,
        nc.semaphore("pe_sem") as pe_sem,
        nc.Block() as block,
    ):

        @block.sync
        def _(sync):
            with nc.allow_non_contiguous_dma(reason="blank-channel gather"):
                sync.dma_start(out=bt[:, 0:H], in_=gsrc[:, 0:H]).then_inc(ga_sem, 16)
            sync.wait_ge(dve_sem, 7)
            sync.dma_start(out=out[:, :], in_=st[:, :]).then_inc(out_sem, 16)
            sync.wait_ge(out_sem, 16)  # out DMA landed before NEFF completion

        @block.scalar
        def _(scalar):
            with nc.allow_non_contiguous_dma(reason="blank-channel gather"):
                scalar.dma_start(out=bt[:, H:KC], in_=gsrc[:, H:KC]).then_inc(
                    gb_sem, 16
                )

        @block.gpsimd
        def _(gpsimd):
            gpsimd.dma_start(out=mt[:, :], in_=msrc[:, :]).then_inc(mk_sem, 16)
            gpsimd.dma_start(out=at[:, :], in_=asrc[:, :]).then_inc(al_sem, 16)

        @block.vector
        def _(vector):
            for b in range(BPC):
                for c in range(BPC):
                    vector.memset(
                        seg[b * PPS : (b + 1) * PPS, c : c + 1],
                        1.0 if b == c else 0.0,
                    ).then_inc(dve_sem, 1)  # -> 4
            vector.wait_ge(al_sem, 16)  # alpha loaded (while gathers fly)
            vector.tensor_reduce(
                red[:, 1:2], at[:, :], mybir.AxisListType.X, mybir.AluOpType.add,
            ).then_inc(dve_sem, 1)  # -> 5
            vector.wait_ge(ga_sem, 16)
            vector.wait_ge(gb_sem, 16)
            vector.wait_ge(mk_sem, 16)
            # spikes = (x < TRIG_C) * mask; accum_out = per-partition counts
            vector.scalar_tensor_tensor(
                jt[:, :], bt[:, :], TRIG_C, mt[:, :],
                mybir.AluOpType.is_lt, mybir.AluOpType.mult,
                accum_out=red[:, 0:1],
            ).then_inc(dve_sem, 1)  # -> 6
            vector.wait_ge(pe_sem, 1)
            vector.tensor_copy(st[:, :], ps[:, :]).then_inc(dve_sem, 1)  # -> 7

        @block.tensor
        def _(tensor):
            tensor.wait_ge(dve_sem, 6)
            # ps[s, j] = sum_p seg[p, s] * red[p, j] (segmented partition sum)
            tensor.matmul(
                ps[:, :], seg[:, :], red[:, :], start=True, stop=True
            ).then_inc(pe_sem, 1)

    return nc


def _device_stats(ctc_log_probs, alpha, mask, trace=False, return_res=False):
    """Run the SPMD bass kernel; returns (n_seg[B], rsum[B], exec_time_ns)."""
    if "nc" not in _CACHE:
        _CACHE["nc"] = build_nc()
    nc = _CACHE["nc"]

    in_maps = []
    for i in range(N_CORES):
        s = slice(i * BPC, (i + 1) * BPC)
        in_maps.append(
            {
                "ctc": np.ascontiguousarray(ctc_log_probs[s], dtype=np.float32),
                "alpha": np.ascontiguousarray(alpha[s], dtype=np.float32),
                "mask": np.ascontiguousarray(mask[s], dtype=np.float32),
            }
        )
    res = run_bass_kernel_spmd(nc, in_maps, list(range(N_CORES)), trace=trace)
    stats = np.concatenate([np.asarray(r["out"]) for r in res.results], axis=0)
    if return_res:
        return stats[:, 0], stats[:, 1], res.exec_time_ns, res
    return stats[:, 0], stats[:, 1], res.exec_time_ns


def _tail(n_seg, rsum, text_length):
    """O(B) scalar tail: combine per-sample stats into the loss."""
    n_seg = n_seg.astype(np.float64)
    rsum = rsum.astype(np.float64)
    text = np.asarray(text_length).astype(np.float64)
    len_i = np.maximum(n_seg, 1.0)
    L = min(len_i.max(), text.max())
    c = np.minimum(text, L)
    loss = np.minimum(n_seg, c) * np.abs(rsum - 1.0) + np.maximum(c - len_i, 0.0)
    return np.float32(loss.sum() / n_seg.shape[0])


def kernel(alpha, ctc_log_probs, mask, text_length):
    alpha = np.asarray(alpha)
    ctc_log_probs = np.asarray(ctc_log_probs)
    mask = np.asarray(mask)
    text_length = np.asarray(text_length)
    n_seg, rsum, _ = _device_stats(ctc_log_probs, alpha, mask)
    return _tail(n_seg, rsum, text_length)


# revision 19
# speedup vs baseline: 1.2073x; 1.0122x over previous
"""CTC boundary loss v3 kernel for 8 Trainium2 NeuronCores.

Math (derived from the reference, which reduces to per-sample scalars):
  blank  = ctc_log_probs[:, :, 0]                      [B,T]
  trig   = (1.0 - blank) > log(3)                      [B,T]
  n_seg  = sum(trig * mask)  per sample                [B]
  rsum   = sum(alpha, axis=1)                          [B]
  len_i  = max(n_seg, 1)
  L      = min(max_i len_i, max_i text_length_i)
  c_i    = min(text_length_i, L)
  loss_i = min(n_seg_i, c_i) * |rsum_i - 1| + relu(c_i - len_i)
  out    = sum(loss_i) / B

Device (data parallel, 2 samples/core): strided gather of the blank
channel + per-sample reductions -> per-core [2, 2] stats (n_seg, rsum).
Host: gathers 16 stats and folds the O(B) scalar tail.
"""

import sys

import numpy as np

if "/opt/trn_rl_repo" not in sys.path:
    sys.path.insert(0, "/opt/trn_rl_repo")

import concourse.bass as bass
import concourse.mybir as mybir
from concourse.bass_utils import run_bass_kernel_spmd

B, T, V = 16, 2048, 1024
N_CORES = 8
BPC = B // N_CORES            # samples per core = 2
P = 128                       # SBUF partitions
PPS = P // BPC                # partitions per sample = 64
KC = T // PPS                 # free-dim cols per partition = 32
LOG_THR = float(np.log(3.0))
# Boundary constant: for every float32 x (incl. +-inf, NaN),
#   (float32(1.0) - x) > float32(LOG_THR)   <=>   x < TRIG_C
# (verified exhaustively around the flip point; it is 2 ulps away from the
# naive 1 - LOG_THR, so the comparison must use this exact constant).
TRIG_C = float(np.float32(-0.09861236810684204))

_CACHE = {}


def build_nc():
    """Raw bass (manual semaphores): single-wait-per-instruction codegen
    rejects Tile's multi-wait joins, and raw bass also skips Tile's
    end-of-kernel all-engine barriers.

    Layout: sample b -> partitions [b*64, (b+1)*64), t = p_local*32 + k.
    The blank-channel gather (4096 4-byte strided descriptors) is split
    across both HWDGE rings (SP + ACT) to halve serial descriptor issue.
    Per-partition partials land in red[128, 2] (spike counts | alpha
    sums); one tiny TensorE matmul against a one-hot segment matrix does
    the segmented partition reduction to [2, 2]."""
    f32 = mybir.dt.float32
    nc = bass.Bass(enable_partition_id=False)
    ctc = nc.dram_tensor("ctc", [BPC, T, V], f32, kind="ExternalInput")
    alpha = nc.dram_tensor("alpha", [BPC, T], f32, kind="ExternalInput")
    mask = nc.dram_tensor("mask", [BPC, T], f32, kind="ExternalInput")
    out = nc.dram_tensor("out", [BPC, 2], f32, kind="ExternalOutput")

    # gather column splits: a small head chunk per ring lets SDMA start
    # draining while the DGE still generates the big chunk's descriptors
    C0, C1, C2, C3 = 0, 4, KC // 2, KC // 2 + 4
    # block layout: sample b -> partitions [b*PPS, (b+1)*PPS), t = p_local*KC + k
    gsrc = ctc[:, :, 0].rearrange("b (p k) -> (b p) k", k=KC)
    asrc = alpha.rearrange("b (p k) -> (b p) k", k=KC)
    msrc = mask.rearrange("b (p k) -> (b p) k", k=KC)

    with (
        nc.sbuf_tensor([P, KC], f32) as bt,    # blank log-probs, gathered
        nc.sbuf_tensor([P, KC], f32) as mt,    # mask
        nc.sbuf_tensor([P, KC], f32) as at,    # alpha
        nc.sbuf_tensor([P, KC], f32) as jt,    # spikes scratch
        nc.sbuf_tensor([P, 2], f32) as red,    # [spike partials | alpha partials]
        nc.sbuf_tensor([P, BPC], f32) as seg,  # one-hot segment matrix
        nc.sbuf_tensor([BPC, 2], f32) as st,   # per-sample [n_seg, rsum]
        nc.psum_tensor([BPC, 2], f32) as ps,
        nc.semaphore("in_sem") as in_sem,
        nc.semaphore("mk_sem") as mk_sem,
        nc.semaphore("al_sem") as al_sem,
        nc.semaphore("out_sem") as out_sem,
        nc.semaphore("dve_sem") as dve_sem,
        nc.semaphore("pe_sem") as pe_sem,
        nc.Block() as block,
    ):

        @block.sync
        def _(sync):
            with nc.allow_non_contiguous_dma(reason="blank-channel gather"):
                sync.dma_start(out=bt[:, C0:C1], in_=gsrc[:, C0:C1]).then_inc(
                    in_sem, 16
                )
                sync.dma_start(out=bt[:, C1:C2], in_=gsrc[:, C1:C2]).then_inc(
                    in_sem, 16
                )
            sync.wait_ge(dve_sem, 7)
            sync.dma_start(out=out[:, :], in_=st[:, :]).then_inc(out_sem, 16)
            sync.wait_ge(out_sem, 16)  # out DMA landed before NEFF completion

        @block.scalar
        def _(scalar):
            with nc.allow_non_contiguous_dma(reason="blank-channel gather"):
                scalar.dma_start(out=bt[:, C2:C3], in_=gsrc[:, C2:C3]).then_inc(
                    in_sem, 16
                )
                scalar.dma_start(out=bt[:, C3:KC], in_=gsrc[:, C3:KC]).then_inc(
                    in_sem, 16
                )

        @block.gpsimd
        def _(gpsimd):
            gpsimd.dma_start(out=mt[:, :], in_=msrc[:, :]).then_inc(mk_sem, 16)
            gpsimd.dma_start(out=at[:, :], in_=asrc[:, :]).then_inc(al_sem, 16)

        @block.vector
        def _(vector):
            for b in range(BPC):
                for c in range(BPC):
                    vector.memset(
                        seg[b * PPS : (b + 1) * PPS, c : c + 1],
                        1.0 if b == c else 0.0,
                    ).then_inc(dve_sem, 1)  # -> 4
            vector.wait_ge(al_sem, 16)  # alpha loaded (while gathers fly)
            vector.tensor_reduce(
                red[:, 1:2], at[:, :], mybir.AxisListType.X, mybir.AluOpType.add,
            ).then_inc(dve_sem, 1)  # -> 5
            vector.wait_ge(in_sem, 64)  # all 4 gather chunks
            vector.wait_ge(mk_sem, 16)  # mask
            # spikes = (x < TRIG_C) * mask; accum_out = per-partition counts
            vector.scalar_tensor_tensor(
                jt[:, :], bt[:, :], TRIG_C, mt[:, :],
                mybir.AluOpType.is_lt, mybir.AluOpType.mult,
                accum_out=red[:, 0:1],
            ).then_inc(dve_sem, 1)  # -> 6
            vector.wait_ge(pe_sem, 1)
            vector.tensor_copy(st[:, :], ps[:, :]).then_inc(dve_sem, 1)  # -> 7

        @block.tensor
        def _(tensor):
            tensor.wait_ge(dve_sem, 6)
            # ps[s, j] = sum_p seg[p, s] * red[p, j] (segmented partition sum)
            tensor.matmul(
                ps[:, :], seg[:, :], red[:, :], start=True, stop=True
            ).then_inc(pe_sem, 1)

    return nc


def _device_stats(ctc_log_probs, alpha, mask, trace=False, return_res=False):
    """Run the SPMD bass kernel; returns (n_seg[B], rsum[B], exec_time_ns)."""
    if "nc" not in _CACHE:
        _CACHE["nc"] = build_nc()
    nc = _CACHE["nc"]

    in_maps = []
    for i in range(N_CORES):
        s = slice(i * BPC, (i + 1) * BPC)
        in_maps.append(
            {
                "ctc": np.ascontiguousarray(ctc_log_probs[s], dtype=np.float32),
                "alpha": np.ascontiguousarray(alpha[s], dtype=np.float32),
                "mask": np.ascontiguousarray(mask[s], dtype=np.float32),
            }
        )
    res = run_bass_kernel_spmd(nc, in_maps, list(range(N_CORES)), trace=trace)
    stats = np.concatenate([np.asarray(r["out"]) for r in res.results], axis=0)
    if return_res:
        return stats[:, 0], stats[:, 1], res.exec_time_ns, res
    return stats[:, 0], stats[:, 1], res.exec_time_ns


def _tail(n_seg, rsum, text_length):
    """O(B) scalar tail: combine per-sample stats into the loss."""
    n_seg = n_seg.astype(np.float64)
    rsum = rsum.astype(np.float64)
    text = np.asarray(text_length).astype(np.float64)
    len_i = np.maximum(n_seg, 1.0)
    L = min(len_i.max(), text.max())
    c = np.minimum(text, L)
    loss = np.minimum(n_seg, c) * np.abs(rsum - 1.0) + np.maximum(c - len_i, 0.0)
    return np.float32(loss.sum() / n_seg.shape[0])


def kernel(alpha, ctc_log_probs, mask, text_length):
    alpha = np.asarray(alpha)
    ctc_log_probs = np.asarray(ctc_log_probs)
    mask = np.asarray(mask)
    text_length = np.asarray(text_length)
    n_seg, rsum, _ = _device_stats(ctc_log_probs, alpha, mask)
    return _tail(n_seg, rsum, text_length)
